# revision 54
# baseline (speedup 1.0000x reference)
"""Trainium2 Bass kernel for nn_MixedAttention (ConvBERT-style mixed attention).

Sharding (key-split data parallel): core = 2*b + j owns KEY/VALUE chunk j
(1024 rows) of batch b plus the conv branch for those rows.  It computes
UNNORMALIZED attention partials for ALL 2048 query rows of batch b over its
1024 keys (numerator ctx^T [64, 2048] per head plus the softmax denominator
via an appended ones-column in v), and the host sums the two cores' partials
and divides.  This removes the k/v double-compute of a query-split scheme,
all on-chip softmax normalization, and every output transpose (host
transposes the [d, s] partials while assembling).

Per-core layout: xT (hidden on partitions) drives every projection.  q is
projected for the full 2048 rows ([own chunk | other chunk] order - host
unpermutes), k/v/co only for the own chunk.  Scores S^T = kT.T @ qT per
128-key block, exp on ACT (scale 1/8 folded), ctx^T accumulated via
lhsT=[v_h | ones] so the denominator falls out as row 64; partials stream
out raw.

Engine assignment (measured op rates on this hw: plain tensor_scalar 4x,
tensor_tensor 2x, scalar_tensor_tensor only 1x): depthwise conv = 9
tensor_scalar_mul taps (4x) + tensor_tensor add tree (2x) on DVE; windowed
einsum = tensor_tensor mul/add, tiles 0-1 on DVE and tile 2 on Pool (which
cannot touch PSUM, so all PSUM evacs stay on DVE).  The conv branch is
emitted early (it only needs x/q-own) so its einsum drains mid-kernel and
the kernel tail is pure attention.  A junk-matmul burst at the start keeps
the PE HAM clock gate from running the first real matmuls at half clock.
"""

import sys

for _p in ("/opt/trn_rl_repo",):
    if _p not in sys.path:
        sys.path.insert(0, _p)

import numpy as np
import ml_dtypes

HIDDEN = 768
N_HEADS = 6
HEAD_DIM = 64
ALL_HEAD = 384
K = 9
B, S = 4, 2048
CHUNK = 1024          # key rows per core
N_CORES = 8
BF16 = ml_dtypes.bfloat16

_COMPILED = {}


def _build_program():
    import concourse.bass as bass
    import concourse.mybir as mybir
    import concourse.tile as tile
    from concourse import bacc
    from contextlib import ExitStack

    dt = mybir.dt
    Alu = mybir.AluOpType
    Act = mybir.ActivationFunctionType

    nc = bacc.Bacc("TRN2", target_bir_lowering=False, debug=False)

    # ---------------- DRAM I/O (host pre-laid in SBUF layout) ----------------
    def din(name, shape, dtype=dt.bfloat16):
        return nc.dram_tensor(name, list(shape), dtype, kind="ExternalInput").ap()

    x_own = din("x_own", [128, 6 * 1032])             # xT own chunk +-4 (padded)
    x_oth = din("x_oth", [128, 6 * 1024])             # xT other chunk
    wk = din("wk", [128, 6 * ALL_HEAD])
    wv = din("wv", [128, 6 * ALL_HEAD])
    wq = din("wq", [128, 6 * ALL_HEAD])
    wco = din("wco", [128, 6 * ALL_HEAD])
    wpw = din("wpw", [128, 6 * ALL_HEAD])
    wck = din("wck", [128, 3 * 54])
    sel = din("sel", [54, 6])                          # head-sum selector
    bvrow = din("bvrow", [1, ALL_HEAD])
    comask = din("comask", [1, 1032])
    bq = din("bq", [128, 3], dt.float32)
    bk = din("bk", [128, 3], dt.float32)
    convb = din("convb", [128, 3], dt.float32)
    bco = din("bco", [128, 3], dt.float32)
    bck = din("bck", [54, 1], dt.float32)
    dwsc = din("dwsc", [128, 6 * K], dt.float32)      # depthwise scalars

    attn = nc.dram_tensor("attn", [65, 6 * S], dt.bfloat16,
                          kind="ExternalOutput").ap()
    conv = nc.dram_tensor("conv", [128, 3 * CHUNK], dt.bfloat16,
                          kind="ExternalOutput").ap()
    pck_dram = nc.dram_tensor("pck_scratch", [54, CHUNK], dt.bfloat16).ap()
    den_dram = nc.dram_tensor("den_scratch", [6, CHUNK], dt.float32).ap()
    denb_dram = nc.dram_tensor("denb_scratch", [6, CHUNK], dt.bfloat16).ap()

    attn_r = attn.rearrange("p (h s) -> p h s", h=6)
    conv_r = conv.rearrange("p (a s) -> p a s", a=3)

    with tile.TileContext(nc) as tc, ExitStack() as ctx:
        singles = ctx.enter_context(tc.tile_pool(name="singles", bufs=1))
        persist = ctx.enter_context(tc.tile_pool(name="persist", bufs=1))
        work = ctx.enter_context(tc.tile_pool(name="work", bufs=3))

        def load(pool, src, shape, dtype=dt.bfloat16, name=None):
            t = pool.tile(shape, dtype, name=name)
            nc.sync.dma_start(out=t, in_=src)
            return t

        # ---------------- load inputs (issue order = priority) --------------
        xosb = singles.tile([128, 6, 1032], dt.bfloat16, name="xosb")
        xtsb = singles.tile([128, 6, 1024], dt.bfloat16, name="xtsb")
        wk_sb = singles.tile([128, 6, ALL_HEAD], dt.bfloat16, name="wk_sb")
        wv_sb = singles.tile([128, 6, ALL_HEAD], dt.bfloat16, name="wv_sb")
        wq_sb = singles.tile([128, 6, ALL_HEAD], dt.bfloat16, name="wq_sb")
        wco_sb = singles.tile([128, 6, ALL_HEAD], dt.bfloat16, name="wco_sb")
        wpw_sb = singles.tile([128, 6, ALL_HEAD], dt.bfloat16, name="wpw_sb")

        xo = x_own.rearrange("p (h s) -> p h s", h=6)
        xt = x_oth.rearrange("p (h s) -> p h s", h=6)
        wkr = wk.rearrange("p (h a) -> p h a", h=6)
        wvr = wv.rearrange("p (h a) -> p h a", h=6)
        wqr = wq.rearrange("p (h a) -> p h a", h=6)
        wcor = wco.rearrange("p (h a) -> p h a", h=6)
        wpwr = wpw.rearrange("p (h a) -> p h a", h=6)

        for dh in range(6):
            nc.sync.dma_start(out=wk_sb[:, dh], in_=wkr[:, dh])
            nc.sync.dma_start(out=xosb[:, dh], in_=xo[:, dh])
        bk_sb = load(singles, bk, [128, 3], dt.float32, name="bk_sb")
        for dh in range(6):
            nc.sync.dma_start(out=wv_sb[:, dh], in_=wvr[:, dh])
        bv_sb = load(singles, bvrow, [1, ALL_HEAD], name="bv_sb")
        for dh in range(6):
            nc.sync.dma_start(out=wq_sb[:, dh], in_=wqr[:, dh])
        bq_sb = load(singles, bq, [128, 3], dt.float32, name="bq_sb")
        dwsc_sb = load(singles, dwsc, [128, 6, K], dt.float32, name="dwsc_sb")
        for dh in range(6):
            nc.sync.dma_start(out=xtsb[:, dh], in_=xt[:, dh])
        for dh in range(6):
            nc.sync.dma_start(out=wco_sb[:, dh], in_=wcor[:, dh])
        bco_sb = load(singles, bco, [128, 3], dt.float32, name="bco_sb")
        mask_sb = singles.tile([128, 1032], dt.bfloat16, name="mask_sb")
        nc.sync.dma_start(out=mask_sb, in_=comask.to_broadcast([128, 1032]))
        for dh in range(6):
            nc.sync.dma_start(out=wpw_sb[:, dh], in_=wpwr[:, dh])
        convb_sb = load(singles, convb, [128, 3], dt.float32, name="convb_sb")
        wck_sb = load(singles, wck, [128, 3, 54], name="wck_sb")
        bck_sb = load(singles, bck, [54, 1], dt.float32, name="bck_sb")
        sel_sb = load(singles, sel, [54, 6], name="sel_sb")

        ones_sb = singles.tile([1, 128], dt.bfloat16, name="ones_sb")
        nc.gpsimd.memset(ones_sb, 1.0)

        # persistent intermediates
        qT = persist.tile([128, 3, S], dt.bfloat16, name="qT")
        kT = persist.tile([128, 3, CHUNK], dt.bfloat16, name="kT")
        vsb = persist.tile([128, 8, 6, 65], dt.bfloat16, name="vsb")
        dwT = persist.tile([128, 6, CHUNK], dt.bfloat16, name="dwT")
        caT = persist.tile([128, 3, CHUNK], dt.bfloat16, name="caT")
        coT = persist.tile([128, 3, 1032], dt.bfloat16, name="coT")
        accT = persist.tile([128, 3, CHUNK], dt.bfloat16, name="accT")
        pck = persist.tile([54, CHUNK], dt.bfloat16, name="pck")
        recipT = persist.tile([6, CHUNK], dt.float32, name="recipT")
        nc.gpsimd.memset(vsb[:, :, :, 64:65], 1.0)

        # PSUM pools: pj (projections) 2 banks, sc (scores) 4, cps (ctx) 2.
        pj = ctx.enter_context(tc.tile_pool(name="psum_pj", bufs=1,
                                            space="PSUM"))
        pa = ctx.enter_context(tc.tile_pool(name="psum_at", bufs=1,
                                            space="PSUM"))

        # HAM warm-up: ~6us of junk matmuls during the input DMA lead-in so
        # the PE clock gate is at 8/8 when the first real matmul issues.
        warm_ps = pj.tile([128, 32], dt.float32, tag="pj", bufs=2,
                          name="warm_ps")
        for _ in range(30):
            nc.tensor.matmul(warm_ps, ones_sb, ones_sb[:, 0:32],
                             start=True, stop=True)

        # ---------------- projection helpers (PE) ----------------------------
        def k_block(at):
            for sb in range(2):
                ps = pj.tile([128, 512], dt.float32, tag="pj", bufs=2,
                             name="pk")
                for dh in range(6):
                    nc.tensor.matmul(
                        ps, wk_sb[:, dh, at * 128:(at + 1) * 128],
                        xosb[:, dh, 4 + sb * 512: 4 + (sb + 1) * 512],
                        start=(dh == 0), stop=(dh == 5))
                nc.vector.tensor_scalar_add(
                    kT[:, at, sb * 512:(sb + 1) * 512], ps, bk_sb[:, at:at + 1])

        def v_block(st):
            pv = pj.tile([128, ALL_HEAD], dt.float32, tag="pj", bufs=2,
                         name="pv")
            for dh in range(6):
                nc.tensor.matmul(
                    pv, xosb[:, dh, 4 + st * 128: 4 + (st + 1) * 128],
                    wv_sb[:, dh, :], start=(dh == 0), stop=False)
            nc.tensor.matmul(pv, ones_sb, bv_sb, start=False, stop=True)
            nc.vector.tensor_copy(vsb[:, st, :, 0:64], pv.rearrange(
                "p (h d) -> p h d", h=6))

        def q_block(at, half, sb):
            # half 0: own chunk -> qT cols [0, 1024); half 1: other chunk.
            ps = pj.tile([128, 512], dt.float32, tag="pj", bufs=2, name="pq")
            for dh in range(6):
                if half == 0:
                    xs = xosb[:, dh, 4 + sb * 512: 4 + (sb + 1) * 512]
                else:
                    xs = xtsb[:, dh, sb * 512:(sb + 1) * 512]
                nc.tensor.matmul(
                    ps, wq_sb[:, dh, at * 128:(at + 1) * 128], xs,
                    start=(dh == 0), stop=(dh == 5))
            nc.vector.tensor_scalar_add(
                qT[:, at, half * 1024 + sb * 512: half * 1024 + (sb + 1) * 512],
                ps, bq_sb[:, at:at + 1])

        # co projection on chunk+-4 (1032 cols), bias + OOB mask on evac
        def co_block(at):
            for (o, w) in ((0, 512), (512, 512), (1024, 8)):
                pco = pj.tile([128, 512], dt.float32, tag="pj", bufs=2,
                              name="pco")
                for dh in range(6):
                    nc.tensor.matmul(
                        pco[:, :w], wco_sb[:, dh, at * 128:(at + 1) * 128],
                        xosb[:, dh, o:o + w],
                        start=(dh == 0), stop=(dh == 5))
                nc.vector.scalar_tensor_tensor(
                    out=coT[:, at, o:o + w], in0=pco[:, :w],
                    scalar=bco_sb[:, at:at + 1], in1=mask_sb[:, o:o + w],
                    op0=Alu.add, op1=Alu.mult)

        # pointwise conv fused with conv_attn: caT = (pw@dwT + convb) * qT
        def pw_block(at):
            for sb in range(2):
                ppw = pj.tile([128, 512], dt.float32, tag="pj", bufs=2,
                              name="ppw")
                for dh in range(6):
                    nc.tensor.matmul(
                        ppw, wpw_sb[:, dh, at * 128:(at + 1) * 128],
                        dwT[:, dh, sb * 512:(sb + 1) * 512],
                        start=(dh == 0), stop=(dh == 5))
                nc.vector.scalar_tensor_tensor(
                    out=caT[:, at, sb * 512:(sb + 1) * 512], in0=ppw,
                    scalar=convb_sb[:, at:at + 1],
                    in1=qT[:, at, sb * 512:(sb + 1) * 512],
                    op0=Alu.add, op1=Alu.mult)

        # depthwise conv: taps via tensor_scalar_mul on DVE (4x mode), add
        # tree on Pool (tensor_tensor, which Pool runs at DVE-2x speed).
        # scalar_tensor_tensor would be one pass per tap but runs at 1x.
        def dw_chain(ct):
            taps = []
            for k in range(K):
                tp = work.tile([128, CHUNK], dt.bfloat16, tag="dwtap", bufs=9,
                               name="dwtap")
                nc.vector.tensor_scalar_mul(
                    tp, xosb[:, ct, k:k + 1024], dwsc_sb[:, ct, k:k + 1])
                taps.append(tp)
            # add tree: 9 -> 5 -> 3 -> 2 -> 1 (last add lands in dwT)
            while len(taps) > 1:
                nxt = []
                for i in range(0, len(taps) - 1, 2):
                    dst = taps[i] if len(taps) > 2 else dwT[:, ct, :]
                    nc.vector.tensor_add(dst, taps[i], taps[i + 1])
                    nxt.append(dst)
                if len(taps) % 2:
                    nxt.append(taps[-1])
                taps = nxt

        # conv kernel layer -> exp -> DRAM (unnormalized; the softmax
        # denominator is divided out of accT at the end of the einsum, so
        # the span-weight broadcast does not wait on the recip roundtrip).
        rcbs = {}

        def ckl_all():
            for sb in range(2):
                pck_ps = pj.tile([54, 512], dt.float32, tag="pj", bufs=2,
                                 name="pck_ps")
                for at in range(3):
                    nc.tensor.matmul(
                        pck_ps, wck_sb[:, at, :],
                        caT[:, at, sb * 512:(sb + 1) * 512],
                        start=(at == 0), stop=(at == 2))
                nc.scalar.activation(pck[:, sb * 512:(sb + 1) * 512], pck_ps,
                                     Act.Exp, bias=bck_sb, scale=1.0)
            nc.sync.dma_start(out=pck_dram, in_=pck)
            for sb in range(2):
                pdt = pj.tile([6, 512], dt.float32, tag="pj", bufs=2,
                              name="pdt")
                nc.tensor.matmul(
                    pdt, sel_sb, pck[:, sb * 512:(sb + 1) * 512],
                    start=True, stop=True)
                nc.vector.reciprocal(recipT[:, sb * 512:(sb + 1) * 512], pdt)
            nc.sync.dma_start(out=den_dram, in_=recipT)
            recipb = persist.tile([6, CHUNK], dt.bfloat16, name="recipb")
            nc.vector.tensor_copy(recipb, recipT)
            nc.sync.dma_start(out=denb_dram, in_=recipb)
            for at in range(3):
                rcb = work.tile([128, CHUNK], dt.bfloat16, tag="rcb", bufs=3,
                                name="rcb")
                for hh in range(2):
                    srcap = bass.AP(
                        tensor=denb_dram.tensor,
                        offset=(2 * at + hh) * CHUNK,
                        ap=[[0, 64], [1, CHUNK]])
                    nc.sync.dma_start(out=rcb[hh * 64:(hh + 1) * 64],
                                      in_=srcap)
                rcbs[at] = rcb

        # span-weight broadcast tiles: 64-way partition broadcast from DRAM
        ckbs = {}

        def ckb_load(at, eng=None):
            eng = eng or nc.sync
            for k in range(K):
                ckb = work.tile([128, CHUNK], dt.bfloat16, tag="ckb", bufs=12,
                                name="ckb")
                for hh in range(2):
                    row = 18 * at + 9 * hh + k
                    srcap = bass.AP(
                        tensor=pck_dram.tensor,
                        offset=row * CHUNK,
                        ap=[[0, 64], [1, CHUNK]])
                    eng.dma_start(out=ckb[hh * 64:(hh + 1) * 64],
                                  in_=srcap)
                ckbs[(at, k)] = ckb

        # windowed einsum: accT[:,at,:] = sum_k ckb_k * coT[:,at,k:] as
        # tensor_tensor mul/add (2x on DVE; Pool takes tile 2).  Split into
        # halves so attention cps evacs interleave in the DVE FIFO.
        def einsum_nom(at, eng, half=None):
            tagsuf = "v" if eng is nc.vector else "g"
            ks = range(0, K) if half is None else (
                range(0, 5) if half == 0 else range(5, K))
            for k in ks:
                if k == 0:
                    eng.tensor_mul(accT[:, at, :], ckbs[(at, 0)],
                                   coT[:, at, 0:CHUNK])
                    continue
                tmp = work.tile([128, CHUNK], dt.bfloat16, tag="tmp" + tagsuf,
                                bufs=2, name="tmp")
                eng.tensor_mul(tmp, ckbs[(at, k)], coT[:, at, k:k + CHUNK])
                eng.tensor_add(accT[:, at, :], tmp, accT[:, at, :])

        def einsum_ship(at):
            nc.vector.tensor_mul(accT[:, at, :], accT[:, at, :], rcbs[at])
            nc.sync.dma_start(out=conv_r[:, at], in_=accT[:, at, :])

        # ---------------- attention: one (head, qchunk) tile ------------------
        def attn_tile(h, qc, inject=None):
            at, lo = h // 2, (h % 2) * 64
            cps = pa.tile([65, 1024], dt.float32, tag="cps", bufs=1,
                          name="cps")
            for sk in range(8):
                if inject and sk in inject:
                    inject[sk]()
                sc = pa.tile([128, 1024], dt.float32, tag="sc", bufs=2,
                             name="sc")
                for qb in range(2):
                    nc.tensor.matmul(
                        sc[:, qb * 512:(qb + 1) * 512],
                        kT[lo:lo + 64, at, sk * 128:(sk + 1) * 128],
                        qT[lo:lo + 64, at,
                           qc * 1024 + qb * 512: qc * 1024 + (qb + 1) * 512],
                        start=True, stop=True)
                pt = work.tile([128, 1024], dt.bfloat16, tag="pt", bufs=4,
                               name="pt")
                nc.scalar.activation(pt, sc, Act.Exp, scale=0.125)
                for qb in range(2):
                    nc.tensor.matmul(
                        cps[:, qb * 512:(qb + 1) * 512], vsb[:, sk, h, :],
                        pt[:, qb * 512:(qb + 1) * 512],
                        start=(sk == 0), stop=(sk == 7))
            # evacuate numerator + denominator row (row 64) in bf16
            stg = work.tile([65, 1024], dt.bfloat16, tag="stg", bufs=6,
                            name="stg")
            nc.scalar.copy(stg, cps)
            nc.sync.dma_start(out=attn_r[:, h, qc * 1024:(qc + 1) * 1024],
                              in_=stg)

        # ---------------- emission schedule ----------------------------------
        # Conv branch work is spread between the early attention tiles (one
        # dw chain per tile keeps each tile's cps evac from queueing behind
        # bulk DVE work); the einsum drains mid-kernel on DVE+Pool so the
        # kernel tail is pure attention.
        k_block(0)
        for st in range(8):
            v_block(st)
        q_block(0, 0, 0)
        q_block(0, 0, 1)
        dw_chain(0)

        attn_tile(0, 0)
        k_block(1)
        q_block(1, 0, 0)
        q_block(1, 0, 1)
        dw_chain(1)
        attn_tile(1, 0)
        k_block(2)
        q_block(2, 0, 0)
        q_block(2, 0, 1)
        dw_chain(2)
        attn_tile(2, 0)
        q_block(0, 1, 0)
        q_block(0, 1, 1)
        dw_chain(3)
        attn_tile(3, 0)
        q_block(1, 1, 0)
        q_block(1, 1, 1)
        dw_chain(4)
        co_block(0)
        attn_tile(4, 0)
        q_block(2, 1, 0)
        q_block(2, 1, 1)
        dw_chain(5)
        co_block(1)
        attn_tile(5, 0)
        co_block(2)
        pw_block(0)
        pw_block(1)
        pw_block(2)
        ckl_all()
        ckb_load(2)
        einsum_nom(2, nc.gpsimd)
        einsum_ship(2)
        attn_tile(0, 1)
        ckb_load(0)
        attn_tile(1, 1)
        einsum_nom(0, nc.vector)
        einsum_ship(0)
        ckb_load(1)
        attn_tile(2, 1)
        einsum_nom(1, nc.vector)
        einsum_ship(1)
        attn_tile(3, 1)
        attn_tile(4, 1)
        attn_tile(5, 1)

    nc.compile()
    return nc


def _prep_in_maps(inputs):
    x = np.asarray(inputs["x"], np.float32)
    dw = np.asarray(inputs["dw"], np.float32).reshape(HIDDEN, K)

    def sb_layout(wT, ntile):  # [ntile*128, F] -> [128, ntile*F]
        f = wT.shape[1]
        return np.ascontiguousarray(
            wT.reshape(ntile, 128, f).transpose(1, 0, 2).reshape(128, ntile * f))

    def wprep(w):  # [A, HIDDEN] -> bf16 [128, 6*A]
        return sb_layout(np.ascontiguousarray(w.T).astype(BF16), 6)

    # depthwise scalars: dwsc[c', ct*9+k] = dw[ct*128+c', k]
    dwsc = np.ascontiguousarray(
        dw.reshape(6, 128, K).transpose(1, 0, 2).reshape(128, 6 * K),
        np.float32)

    com = {
        "wq": wprep(inputs["Wq"]), "wk": wprep(inputs["Wk"]),
        "wv": wprep(inputs["Wv"]), "wco": wprep(inputs["Wco"]),
        "wpw": wprep(inputs["pw"]),
        "wck": sb_layout(np.ascontiguousarray(inputs["Wck"].T).astype(BF16), 3),
        "sel": np.kron(np.eye(N_HEADS), np.ones((K, 1))).astype(BF16),
        "bvrow": inputs["bv"].reshape(1, ALL_HEAD).astype(BF16),
        "bq": np.ascontiguousarray(inputs["bq"].reshape(3, 128).T, np.float32),
        "bk": np.ascontiguousarray(inputs["bk"].reshape(3, 128).T, np.float32),
        "convb": np.ascontiguousarray(
            inputs["conv_bias"].reshape(3, 128).T, np.float32),
        "bco": np.ascontiguousarray(inputs["bco"].reshape(3, 128).T, np.float32),
        "bck": inputs["bck"].reshape(54, 1).astype(np.float32),
        "dwsc": dwsc,
    }

    in_maps = []
    for b in range(B):
        xb = x[b]                                   # [S, HIDDEN]
        xTb = np.ascontiguousarray(xb.T).astype(BF16)   # [768, S]
        xT_pad = np.zeros((HIDDEN, S + 8), BF16)
        xT_pad[:, 4:4 + S] = xTb
        for j in range(2):
            own = np.ascontiguousarray(xT_pad[:, j * CHUNK: j * CHUNK + 1032])
            oth = np.ascontiguousarray(
                xTb[:, (1 - j) * CHUNK: (2 - j) * CHUNK])
            g0 = j * CHUNK - 4
            mrows = np.arange(g0, g0 + 1032)
            comask = ((mrows >= 0) & (mrows < S)).astype(BF16).reshape(1, 1032)
            m = dict(com)
            m["x_own"] = sb_layout(own, 6)
            m["x_oth"] = sb_layout(oth, 6)
            m["comask"] = comask
            in_maps.append(m)
    return in_maps


def _gather(results):
    out = np.empty((B, S, 2 * ALL_HEAD), np.float32)
    for b in range(B):
        r0, r1 = results[2 * b], results[2 * b + 1]
        # attn partials: [65, 6, 2048] own-chunk-first; row 64 = denominator
        a0 = np.asarray(r0["attn"]).astype(np.float32).reshape(65, 6, S)
        a1 = np.asarray(r1["attn"]).astype(np.float32).reshape(65, 6, S)
        n0, d0 = a0[:64], a0[64]
        n1, d1 = a1[:64], a1[64]
        # core 1's columns are [own=chunk1 | other=chunk0]: swap to global
        n1 = np.concatenate([n1[:, :, CHUNK:], n1[:, :, :CHUNK]], axis=2)
        d1 = np.concatenate([d1[:, CHUNK:], d1[:, :CHUNK]], axis=1)
        ctx = (n0 + n1) / (d0 + d1)[None]            # [64, 6, 2048]
        out[b, :, :ALL_HEAD] = ctx.transpose(2, 1, 0).reshape(S, ALL_HEAD)
        for j in range(2):
            cv = np.asarray(results[2 * b + j]["conv"]).astype(np.float32)
            cv = cv.reshape(128, 3, CHUNK).transpose(2, 1, 0).reshape(
                CHUNK, ALL_HEAD)
            out[b, j * CHUNK:(j + 1) * CHUNK, ALL_HEAD:] = cv
    return out


def kernel(**inputs):
    from concourse.bass_utils import run_bass_kernel_spmd

    key = "prog"
    if key not in _COMPILED:
        _COMPILED[key] = _build_program()
    nc = _COMPILED[key]
    in_maps = _prep_in_maps(inputs)
    res = run_bass_kernel_spmd(nc, in_maps, list(range(N_CORES)))
    return _gather(res.results)


if __name__ == "__main__":
    import reference
    inp = {k: np.asarray(v) for k, v in reference.setup_inputs().items()}
    got = kernel(**inp)
    want = np.asarray(reference.reference(**inp))
    err = np.linalg.norm(got - want) / np.linalg.norm(want)
    print("rel err:", err)


# revision 55
# speedup vs baseline: 1.0206x; 1.0206x over previous
"""Trainium2 Bass kernel for nn_MixedAttention (ConvBERT-style mixed attention).

Sharding (key-split data parallel): core = 2*b + j owns KEY/VALUE chunk j
(1024 rows) of batch b plus the conv branch for those rows.  It computes
UNNORMALIZED attention partials for ALL 2048 query rows of batch b over its
1024 keys (numerator ctx^T [64, 2048] per head plus the softmax denominator
via an appended ones-column in v), and the host sums the two cores' partials
and divides.  This removes the k/v double-compute of a query-split scheme,
all on-chip softmax normalization, and every output transpose (host
transposes the [d, s] partials while assembling).

Per-core layout: xT (hidden on partitions) drives every projection.  q is
projected for the full 2048 rows ([own chunk | other chunk] order - host
unpermutes), k/v/co only for the own chunk.  Scores S^T = kT.T @ qT per
128-key block, exp on ACT (scale 1/8 folded), ctx^T accumulated via
lhsT=[v_h | ones] so the denominator falls out as row 64; partials stream
out raw.

Engine assignment (measured op rates on this hw: plain tensor_scalar 4x,
tensor_tensor 2x, scalar_tensor_tensor only 1x): depthwise conv = 9
tensor_scalar_mul taps (4x) + tensor_tensor add tree (2x) on DVE; windowed
einsum = tensor_tensor mul/add, tiles 0-1 on DVE and tile 2 on Pool (which
cannot touch PSUM, so all PSUM evacs stay on DVE).  The conv branch is
emitted early (it only needs x/q-own) so its einsum drains mid-kernel and
the kernel tail is pure attention.  A junk-matmul burst at the start keeps
the PE HAM clock gate from running the first real matmuls at half clock.
"""

import sys

for _p in ("/opt/trn_rl_repo",):
    if _p not in sys.path:
        sys.path.insert(0, _p)

import numpy as np
import ml_dtypes

HIDDEN = 768
N_HEADS = 6
HEAD_DIM = 64
ALL_HEAD = 384
K = 9
B, S = 4, 2048
CHUNK = 1024          # key rows per core
N_CORES = 8
BF16 = ml_dtypes.bfloat16

_COMPILED = {}


def _build_program():
    import concourse.bass as bass
    import concourse.mybir as mybir
    import concourse.tile as tile
    from concourse import bacc
    from contextlib import ExitStack

    dt = mybir.dt
    Alu = mybir.AluOpType
    Act = mybir.ActivationFunctionType

    nc = bacc.Bacc("TRN2", target_bir_lowering=False, debug=False)

    # ---------------- DRAM I/O (host pre-laid in SBUF layout) ----------------
    def din(name, shape, dtype=dt.bfloat16):
        return nc.dram_tensor(name, list(shape), dtype, kind="ExternalInput").ap()

    x_own = din("x_own", [128, 6 * 1032])             # xT own chunk +-4 (padded)
    x_oth = din("x_oth", [128, 6 * 1024])             # xT other chunk
    wk = din("wk", [128, 6 * ALL_HEAD])
    wv = din("wv", [128, 6 * ALL_HEAD])
    wq = din("wq", [128, 6 * ALL_HEAD])
    wco = din("wco", [128, 6 * ALL_HEAD])
    wpw = din("wpw", [128, 6 * ALL_HEAD])
    wck = din("wck", [128, 3 * 54])
    sel = din("sel", [54, 6])                          # head-sum selector
    bvrow = din("bvrow", [1, ALL_HEAD])
    comask = din("comask", [1, 1032])
    bq = din("bq", [128, 3], dt.float32)
    bk = din("bk", [128, 3], dt.float32)
    convb = din("convb", [128, 3], dt.float32)
    bco = din("bco", [128, 3], dt.float32)
    bck = din("bck", [54, 1], dt.float32)
    dwsc = din("dwsc", [128, 6 * K], dt.float32)      # depthwise scalars
    dwd = din("dwd", [128, 3 * K * 128])              # diag dw mats (unused)

    attn = nc.dram_tensor("attn", [65, 6 * S], dt.bfloat16,
                          kind="ExternalOutput").ap()
    conv = nc.dram_tensor("conv", [128, 3 * CHUNK], dt.bfloat16,
                          kind="ExternalOutput").ap()
    pck_dram = nc.dram_tensor("pck_scratch", [54, CHUNK], dt.bfloat16).ap()
    den_dram = nc.dram_tensor("den_scratch", [6, CHUNK], dt.float32).ap()
    denb_dram = nc.dram_tensor("denb_scratch", [6, CHUNK], dt.bfloat16).ap()

    attn_r = attn.rearrange("p (h s) -> p h s", h=6)
    conv_r = conv.rearrange("p (a s) -> p a s", a=3)

    with tile.TileContext(nc) as tc, ExitStack() as ctx:
        singles = ctx.enter_context(tc.tile_pool(name="singles", bufs=1))
        persist = ctx.enter_context(tc.tile_pool(name="persist", bufs=1))
        work = ctx.enter_context(tc.tile_pool(name="work", bufs=3))

        def load(pool, src, shape, dtype=dt.bfloat16, name=None):
            t = pool.tile(shape, dtype, name=name)
            nc.sync.dma_start(out=t, in_=src)
            return t

        # ---------------- load inputs (issue order = priority) --------------
        xosb = singles.tile([128, 6, 1032], dt.bfloat16, name="xosb")
        xtsb = singles.tile([128, 6, 1024], dt.bfloat16, name="xtsb")
        wk_sb = singles.tile([128, 6, ALL_HEAD], dt.bfloat16, name="wk_sb")
        wv_sb = singles.tile([128, 6, ALL_HEAD], dt.bfloat16, name="wv_sb")
        wq_sb = singles.tile([128, 6, ALL_HEAD], dt.bfloat16, name="wq_sb")
        wco_sb = singles.tile([128, 6, ALL_HEAD], dt.bfloat16, name="wco_sb")
        wpw_sb = singles.tile([128, 6, ALL_HEAD], dt.bfloat16, name="wpw_sb")

        xo = x_own.rearrange("p (h s) -> p h s", h=6)
        xt = x_oth.rearrange("p (h s) -> p h s", h=6)
        wkr = wk.rearrange("p (h a) -> p h a", h=6)
        wvr = wv.rearrange("p (h a) -> p h a", h=6)
        wqr = wq.rearrange("p (h a) -> p h a", h=6)
        wcor = wco.rearrange("p (h a) -> p h a", h=6)
        wpwr = wpw.rearrange("p (h a) -> p h a", h=6)

        for dh in range(6):
            nc.sync.dma_start(out=wk_sb[:, dh], in_=wkr[:, dh])
            nc.sync.dma_start(out=xosb[:, dh], in_=xo[:, dh])
        bk_sb = load(singles, bk, [128, 3], dt.float32, name="bk_sb")
        for dh in range(6):
            nc.sync.dma_start(out=wv_sb[:, dh], in_=wvr[:, dh])
        bv_sb = load(singles, bvrow, [1, ALL_HEAD], name="bv_sb")
        for dh in range(6):
            nc.sync.dma_start(out=wq_sb[:, dh], in_=wqr[:, dh])
        bq_sb = load(singles, bq, [128, 3], dt.float32, name="bq_sb")
        dwsc_sb = load(singles, dwsc, [128, 6, K], dt.float32, name="dwsc_sb")
        for dh in range(6):
            nc.sync.dma_start(out=xtsb[:, dh], in_=xt[:, dh])
        for dh in range(6):
            nc.sync.dma_start(out=wco_sb[:, dh], in_=wcor[:, dh])
        bco_sb = load(singles, bco, [128, 3], dt.float32, name="bco_sb")
        mask_sb = singles.tile([128, 1032], dt.bfloat16, name="mask_sb")
        nc.sync.dma_start(out=mask_sb, in_=comask.to_broadcast([128, 1032]))
        for dh in range(6):
            nc.sync.dma_start(out=wpw_sb[:, dh], in_=wpwr[:, dh])
        convb_sb = load(singles, convb, [128, 3], dt.float32, name="convb_sb")
        wck_sb = load(singles, wck, [128, 3, 54], name="wck_sb")
        bck_sb = load(singles, bck, [54, 1], dt.float32, name="bck_sb")
        sel_sb = load(singles, sel, [54, 6], name="sel_sb")

        dwd_sb = singles.tile([128, 3, K, 128], dt.bfloat16, name="dwd_sb")
        nc.sync.dma_start(out=dwd_sb, in_=dwd)
        ones_sb = singles.tile([1, 128], dt.bfloat16, name="ones_sb")
        nc.gpsimd.memset(ones_sb, 1.0)

        # persistent intermediates
        qT = persist.tile([128, 3, S], dt.bfloat16, name="qT")
        kT = persist.tile([128, 3, CHUNK], dt.bfloat16, name="kT")
        vsb = persist.tile([128, 8, 6, 65], dt.bfloat16, name="vsb")
        dwT = persist.tile([128, 6, CHUNK], dt.bfloat16, name="dwT")
        caT = persist.tile([128, 3, CHUNK], dt.bfloat16, name="caT")
        coT = persist.tile([128, 3, 1032], dt.bfloat16, name="coT")
        accT = persist.tile([128, 3, CHUNK], dt.bfloat16, name="accT")
        pck = persist.tile([54, CHUNK], dt.bfloat16, name="pck")
        recipT = persist.tile([6, CHUNK], dt.float32, name="recipT")
        nc.gpsimd.memset(vsb[:, :, :, 64:65], 1.0)

        # PSUM pools: pj (projections) 2 banks, sc (scores) 4, cps (ctx) 2.
        pj = ctx.enter_context(tc.tile_pool(name="psum_pj", bufs=1,
                                            space="PSUM"))
        pa = ctx.enter_context(tc.tile_pool(name="psum_at", bufs=1,
                                            space="PSUM"))

        # HAM warm-up: ~6us of junk matmuls during the input DMA lead-in so
        # the PE clock gate is at 8/8 when the first real matmul issues.
        warm_ps = pj.tile([128, 32], dt.float32, tag="pj", bufs=2,
                          name="warm_ps")
        for _ in range(30):
            nc.tensor.matmul(warm_ps, ones_sb, ones_sb[:, 0:32],
                             start=True, stop=True)

        # ---------------- projection helpers (PE) ----------------------------
        def k_block(at):
            for sb in range(2):
                ps = pj.tile([128, 512], dt.float32, tag="pj", bufs=2,
                             name="pk")
                for dh in range(6):
                    nc.tensor.matmul(
                        ps, wk_sb[:, dh, at * 128:(at + 1) * 128],
                        xosb[:, dh, 4 + sb * 512: 4 + (sb + 1) * 512],
                        start=(dh == 0), stop=(dh == 5))
                nc.vector.tensor_scalar_add(
                    kT[:, at, sb * 512:(sb + 1) * 512], ps, bk_sb[:, at:at + 1])

        def v_block(st):
            pv = pj.tile([128, ALL_HEAD], dt.float32, tag="pj", bufs=2,
                         name="pv")
            for dh in range(6):
                nc.tensor.matmul(
                    pv, xosb[:, dh, 4 + st * 128: 4 + (st + 1) * 128],
                    wv_sb[:, dh, :], start=(dh == 0), stop=False)
            nc.tensor.matmul(pv, ones_sb, bv_sb, start=False, stop=True)
            nc.vector.tensor_copy(vsb[:, st, :, 0:64], pv.rearrange(
                "p (h d) -> p h d", h=6))

        def q_block(at, half, sb):
            # half 0: own chunk -> qT cols [0, 1024); half 1: other chunk.
            ps = pj.tile([128, 512], dt.float32, tag="pj", bufs=2, name="pq")
            for dh in range(6):
                if half == 0:
                    xs = xosb[:, dh, 4 + sb * 512: 4 + (sb + 1) * 512]
                else:
                    xs = xtsb[:, dh, sb * 512:(sb + 1) * 512]
                nc.tensor.matmul(
                    ps, wq_sb[:, dh, at * 128:(at + 1) * 128], xs,
                    start=(dh == 0), stop=(dh == 5))
            nc.vector.tensor_scalar_add(
                qT[:, at, half * 1024 + sb * 512: half * 1024 + (sb + 1) * 512],
                ps, bq_sb[:, at:at + 1])

        # co projection on chunk+-4 (1032 cols), bias + OOB mask on evac
        def co_block(at):
            for (o, w) in ((0, 512), (512, 512), (1024, 8)):
                pco = pj.tile([128, 512], dt.float32, tag="pj", bufs=2,
                              name="pco")
                for dh in range(6):
                    nc.tensor.matmul(
                        pco[:, :w], wco_sb[:, dh, at * 128:(at + 1) * 128],
                        xosb[:, dh, o:o + w],
                        start=(dh == 0), stop=(dh == 5))
                nc.vector.scalar_tensor_tensor(
                    out=coT[:, at, o:o + w], in0=pco[:, :w],
                    scalar=bco_sb[:, at:at + 1], in1=mask_sb[:, o:o + w],
                    op0=Alu.add, op1=Alu.mult)

        # pointwise conv fused with conv_attn: caT = (pw@dwT + convb) * qT
        def pw_block(at):
            for sb in range(2):
                ppw = pj.tile([128, 512], dt.float32, tag="pj", bufs=2,
                              name="ppw")
                for dh in range(6):
                    nc.tensor.matmul(
                        ppw, wpw_sb[:, dh, at * 128:(at + 1) * 128],
                        dwT[:, dh, sb * 512:(sb + 1) * 512],
                        start=(dh == 0), stop=(dh == 5))
                nc.vector.scalar_tensor_tensor(
                    out=caT[:, at, sb * 512:(sb + 1) * 512], in0=ppw,
                    scalar=convb_sb[:, at:at + 1],
                    in1=qT[:, at, sb * 512:(sb + 1) * 512],
                    op0=Alu.add, op1=Alu.mult)

        # depthwise conv: taps via tensor_scalar_mul on DVE (4x mode), add
        # tree on Pool (tensor_tensor, which Pool runs at DVE-2x speed).
        # scalar_tensor_tensor would be one pass per tap but runs at 1x.
        def dw_chain(ct):
            taps = []
            for k in range(K):
                tp = work.tile([128, CHUNK], dt.bfloat16, tag="dwtap", bufs=9,
                               name="dwtap")
                nc.vector.tensor_scalar_mul(
                    tp, xosb[:, ct, k:k + 1024], dwsc_sb[:, ct, k:k + 1])
                taps.append(tp)
            # add tree: 9 -> 5 -> 3 -> 2 -> 1 (last add lands in dwT)
            while len(taps) > 1:
                nxt = []
                for i in range(0, len(taps) - 1, 2):
                    dst = taps[i] if len(taps) > 2 else dwT[:, ct, :]
                    nc.vector.tensor_add(dst, taps[i], taps[i + 1])
                    nxt.append(dst)
                if len(taps) % 2:
                    nxt.append(taps[-1])
                taps = nxt

        # conv kernel layer -> exp -> DRAM (unnormalized; the softmax
        # denominator is divided out of accT at the end of the einsum, so
        # the span-weight broadcast does not wait on the recip roundtrip).
        rcbs = {}

        def ckl_all():
            for sb in range(2):
                pck_ps = pj.tile([54, 512], dt.float32, tag="pj", bufs=2,
                                 name="pck_ps")
                for at in range(3):
                    nc.tensor.matmul(
                        pck_ps, wck_sb[:, at, :],
                        caT[:, at, sb * 512:(sb + 1) * 512],
                        start=(at == 0), stop=(at == 2))
                nc.scalar.activation(pck[:, sb * 512:(sb + 1) * 512], pck_ps,
                                     Act.Exp, bias=bck_sb, scale=1.0)
            nc.sync.dma_start(out=pck_dram, in_=pck)
            for sb in range(2):
                pdt = pj.tile([6, 512], dt.float32, tag="pj", bufs=2,
                              name="pdt")
                nc.tensor.matmul(
                    pdt, sel_sb, pck[:, sb * 512:(sb + 1) * 512],
                    start=True, stop=True)
                nc.vector.reciprocal(recipT[:, sb * 512:(sb + 1) * 512], pdt)
            nc.sync.dma_start(out=den_dram, in_=recipT)
            recipb = persist.tile([6, CHUNK], dt.bfloat16, name="recipb")
            nc.vector.tensor_copy(recipb, recipT)
            nc.sync.dma_start(out=denb_dram, in_=recipb)
            for at in range(3):
                rcb = work.tile([128, CHUNK], dt.bfloat16, tag="rcb", bufs=3,
                                name="rcb")
                for hh in range(2):
                    srcap = bass.AP(
                        tensor=denb_dram.tensor,
                        offset=(2 * at + hh) * CHUNK,
                        ap=[[0, 64], [1, CHUNK]])
                    nc.sync.dma_start(out=rcb[hh * 64:(hh + 1) * 64],
                                      in_=srcap)
                rcbs[at] = rcb

        # span-weight broadcast tiles: 64-way partition broadcast from DRAM
        ckbs = {}

        def ckb_load(at, eng=None):
            eng = eng or nc.sync
            for k in range(K):
                ckb = work.tile([128, CHUNK], dt.bfloat16, tag="ckb", bufs=12,
                                name="ckb")
                for hh in range(2):
                    row = 18 * at + 9 * hh + k
                    srcap = bass.AP(
                        tensor=pck_dram.tensor,
                        offset=row * CHUNK,
                        ap=[[0, 64], [1, CHUNK]])
                    eng.dma_start(out=ckb[hh * 64:(hh + 1) * 64],
                                  in_=srcap)
                ckbs[(at, k)] = ckb

        # windowed einsum: accT[:,at,:] = sum_k ckb_k * coT[:,at,k:] as
        # tensor_tensor mul/add (2x on DVE; Pool takes tile 2).  Split into
        # halves so attention cps evacs interleave in the DVE FIFO.
        def einsum_nom(at, eng, half=None):
            tagsuf = "v" if eng is nc.vector else "g"
            ks = range(0, K) if half is None else (
                range(0, 5) if half == 0 else range(5, K))
            for k in ks:
                if k == 0:
                    eng.tensor_mul(accT[:, at, :], ckbs[(at, 0)],
                                   coT[:, at, 0:CHUNK])
                    continue
                tmp = work.tile([128, CHUNK], dt.bfloat16, tag="tmp" + tagsuf,
                                bufs=2, name="tmp")
                eng.tensor_mul(tmp, ckbs[(at, k)], coT[:, at, k:k + CHUNK])
                eng.tensor_add(accT[:, at, :], tmp, accT[:, at, :])

        def einsum_ship(at):
            nc.vector.tensor_mul(accT[:, at, :], accT[:, at, :], rcbs[at])
            nc.sync.dma_start(out=conv_r[:, at], in_=accT[:, at, :])

        # ---------------- attention: one (head, qchunk) tile ------------------
        def attn_tile(h, qc, inject=None):
            at, lo = h // 2, (h % 2) * 64
            cps = pa.tile([65, 1024], dt.float32, tag="cps", bufs=1,
                          name="cps")
            for sk in range(8):
                if inject and sk in inject:
                    inject[sk]()
                sc = pa.tile([128, 1024], dt.float32, tag="sc", bufs=2,
                             name="sc")
                for qb in range(2):
                    nc.tensor.matmul(
                        sc[:, qb * 512:(qb + 1) * 512],
                        kT[lo:lo + 64, at, sk * 128:(sk + 1) * 128],
                        qT[lo:lo + 64, at,
                           qc * 1024 + qb * 512: qc * 1024 + (qb + 1) * 512],
                        start=True, stop=True)
                pt = work.tile([128, 1024], dt.bfloat16, tag="pt", bufs=4,
                               name="pt")
                nc.scalar.activation(pt, sc, Act.Exp, scale=0.125)
                for qb in range(2):
                    nc.tensor.matmul(
                        cps[:, qb * 512:(qb + 1) * 512], vsb[:, sk, h, :],
                        pt[:, qb * 512:(qb + 1) * 512],
                        start=(sk == 0), stop=(sk == 7))
            # evacuate numerator + denominator row (row 64) in bf16
            stg = work.tile([65, 1024], dt.bfloat16, tag="stg", bufs=6,
                            name="stg")
            nc.scalar.copy(stg, cps)
            nc.sync.dma_start(out=attn_r[:, h, qc * 1024:(qc + 1) * 1024],
                              in_=stg)

        # ---------------- emission schedule ----------------------------------
        # Conv branch work is spread between the early attention tiles (one
        # dw chain per tile keeps each tile's cps evac from queueing behind
        # bulk DVE work); the einsum drains mid-kernel on DVE+Pool so the
        # kernel tail is pure attention.
        k_block(0)
        for st in range(8):
            v_block(st)
        q_block(0, 0, 0)
        q_block(0, 0, 1)
        dw_chain(0)

        attn_tile(0, 0)
        k_block(1)
        q_block(1, 0, 0)
        q_block(1, 0, 1)
        dw_chain(1)
        attn_tile(1, 0)
        k_block(2)
        q_block(2, 0, 0)
        q_block(2, 0, 1)
        dw_chain(2)
        attn_tile(2, 0)
        q_block(0, 1, 0)
        q_block(0, 1, 1)
        dw_chain(3)
        attn_tile(3, 0)
        q_block(1, 1, 0)
        q_block(1, 1, 1)
        dw_chain(4)
        co_block(0)
        attn_tile(4, 0)
        q_block(2, 1, 0)
        q_block(2, 1, 1)
        dw_chain(5)
        co_block(1)
        attn_tile(5, 0)
        co_block(2)
        pw_block(0)
        pw_block(1)
        pw_block(2)
        ckl_all()
        ckb_load(2)
        einsum_nom(2, nc.gpsimd)
        einsum_ship(2)
        attn_tile(0, 1)
        ckb_load(0)
        attn_tile(1, 1)
        einsum_nom(0, nc.vector)
        einsum_ship(0)
        ckb_load(1)
        attn_tile(2, 1)
        einsum_nom(1, nc.vector)
        einsum_ship(1)
        attn_tile(3, 1)
        attn_tile(4, 1)
        attn_tile(5, 1)

    nc.compile()
    return nc


def _prep_in_maps(inputs):
    x = np.asarray(inputs["x"], np.float32)
    dw = np.asarray(inputs["dw"], np.float32).reshape(HIDDEN, K)

    def sb_layout(wT, ntile):  # [ntile*128, F] -> [128, ntile*F]
        f = wT.shape[1]
        return np.ascontiguousarray(
            wT.reshape(ntile, 128, f).transpose(1, 0, 2).reshape(128, ntile * f))

    def wprep(w):  # [A, HIDDEN] -> bf16 [128, 6*A]
        return sb_layout(np.ascontiguousarray(w.T).astype(BF16), 6)

    # depthwise scalars: dwsc[c', ct*9+k] = dw[ct*128+c', k]
    dwsc = np.ascontiguousarray(
        dw.reshape(6, 128, K).transpose(1, 0, 2).reshape(128, 6 * K),
        np.float32)

    dwdm = np.zeros((128, 3, K, 128), BF16)
    ii = np.arange(128)
    for ct in (3, 4, 5):
        for k in range(K):
            dwdm[ii, ct - 3, k, ii] = dw[ct * 128 + ii, k].astype(BF16)

    com = {
        "wq": wprep(inputs["Wq"]), "wk": wprep(inputs["Wk"]),
        "wv": wprep(inputs["Wv"]), "wco": wprep(inputs["Wco"]),
        "wpw": wprep(inputs["pw"]),
        "wck": sb_layout(np.ascontiguousarray(inputs["Wck"].T).astype(BF16), 3),
        "sel": np.kron(np.eye(N_HEADS), np.ones((K, 1))).astype(BF16),
        "bvrow": inputs["bv"].reshape(1, ALL_HEAD).astype(BF16),
        "bq": np.ascontiguousarray(inputs["bq"].reshape(3, 128).T, np.float32),
        "bk": np.ascontiguousarray(inputs["bk"].reshape(3, 128).T, np.float32),
        "convb": np.ascontiguousarray(
            inputs["conv_bias"].reshape(3, 128).T, np.float32),
        "bco": np.ascontiguousarray(inputs["bco"].reshape(3, 128).T, np.float32),
        "bck": inputs["bck"].reshape(54, 1).astype(np.float32),
        "dwsc": dwsc,
        "dwd": dwdm.reshape(128, 3 * K * 128),
    }

    in_maps = []
    for b in range(B):
        xb = x[b]                                   # [S, HIDDEN]
        xTb = np.ascontiguousarray(xb.T).astype(BF16)   # [768, S]
        xT_pad = np.zeros((HIDDEN, S + 8), BF16)
        xT_pad[:, 4:4 + S] = xTb
        for j in range(2):
            own = np.ascontiguousarray(xT_pad[:, j * CHUNK: j * CHUNK + 1032])
            oth = np.ascontiguousarray(
                xTb[:, (1 - j) * CHUNK: (2 - j) * CHUNK])
            g0 = j * CHUNK - 4
            mrows = np.arange(g0, g0 + 1032)
            comask = ((mrows >= 0) & (mrows < S)).astype(BF16).reshape(1, 1032)
            m = dict(com)
            m["x_own"] = sb_layout(own, 6)
            m["x_oth"] = sb_layout(oth, 6)
            m["comask"] = comask
            in_maps.append(m)
    return in_maps


def _gather(results):
    out = np.empty((B, S, 2 * ALL_HEAD), np.float32)
    for b in range(B):
        r0, r1 = results[2 * b], results[2 * b + 1]
        # attn partials: [65, 6, 2048] own-chunk-first; row 64 = denominator
        a0 = np.asarray(r0["attn"]).astype(np.float32).reshape(65, 6, S)
        a1 = np.asarray(r1["attn"]).astype(np.float32).reshape(65, 6, S)
        n0, d0 = a0[:64], a0[64]
        n1, d1 = a1[:64], a1[64]
        # core 1's columns are [own=chunk1 | other=chunk0]: swap to global
        n1 = np.concatenate([n1[:, :, CHUNK:], n1[:, :, :CHUNK]], axis=2)
        d1 = np.concatenate([d1[:, CHUNK:], d1[:, :CHUNK]], axis=1)
        ctx = (n0 + n1) / (d0 + d1)[None]            # [64, 6, 2048]
        out[b, :, :ALL_HEAD] = ctx.transpose(2, 1, 0).reshape(S, ALL_HEAD)
        for j in range(2):
            cv = np.asarray(results[2 * b + j]["conv"]).astype(np.float32)
            cv = cv.reshape(128, 3, CHUNK).transpose(2, 1, 0).reshape(
                CHUNK, ALL_HEAD)
            out[b, j * CHUNK:(j + 1) * CHUNK, ALL_HEAD:] = cv
    return out


def kernel(**inputs):
    from concourse.bass_utils import run_bass_kernel_spmd

    key = "prog"
    if key not in _COMPILED:
        _COMPILED[key] = _build_program()
    nc = _COMPILED[key]
    in_maps = _prep_in_maps(inputs)
    res = run_bass_kernel_spmd(nc, in_maps, list(range(N_CORES)))
    return _gather(res.results)


if __name__ == "__main__":
    import reference
    inp = {k: np.asarray(v) for k, v in reference.setup_inputs().items()}
    got = kernel(**inp)
    want = np.asarray(reference.reference(**inp))
    err = np.linalg.norm(got - want) / np.linalg.norm(want)
    print("rel err:", err)


# revision 56
# speedup vs baseline: 1.0574x; 1.0361x over previous
"""Trainium2 Bass kernel for nn_MixedAttention (ConvBERT-style mixed attention).

Sharding (key-split data parallel): core = 2*b + j owns KEY/VALUE chunk j
(1024 rows) of batch b plus the conv branch for those rows.  It computes
UNNORMALIZED attention partials for ALL 2048 query rows of batch b over its
1024 keys (numerator ctx^T [64, 2048] per head plus the softmax denominator
via an appended ones-column in v), and the host sums the two cores' partials
and divides.  This removes the k/v double-compute of a query-split scheme,
all on-chip softmax normalization, and every output transpose (host
transposes the [d, s] partials while assembling).

Per-core layout: xT (hidden on partitions) drives every projection.  q is
projected for the full 2048 rows ([own chunk | other chunk] order - host
unpermutes), k/v/co only for the own chunk.  Scores S^T = kT.T @ qT per
128-key block, exp on ACT (scale 1/8 folded), ctx^T accumulated via
lhsT=[v_h | ones] so the denominator falls out as row 64; partials stream
out raw.

Engine assignment (measured op rates on this hw: plain tensor_scalar 4x,
tensor_tensor 2x, scalar_tensor_tensor only 1x): depthwise conv = 9
tensor_scalar_mul taps (4x) + tensor_tensor add tree (2x) on DVE; windowed
einsum = tensor_tensor mul/add, tiles 0-1 on DVE and tile 2 on Pool (which
cannot touch PSUM, so all PSUM evacs stay on DVE).  The conv branch is
emitted early (it only needs x/q-own) so its einsum drains mid-kernel and
the kernel tail is pure attention.  A junk-matmul burst at the start keeps
the PE HAM clock gate from running the first real matmuls at half clock.
"""

import sys

for _p in ("/opt/trn_rl_repo",):
    if _p not in sys.path:
        sys.path.insert(0, _p)

import numpy as np
import ml_dtypes

HIDDEN = 768
N_HEADS = 6
HEAD_DIM = 64
ALL_HEAD = 384
K = 9
B, S = 4, 2048
CHUNK = 1024          # key rows per core
N_CORES = 8
BF16 = ml_dtypes.bfloat16

_COMPILED = {}


def _build_program():
    import concourse.bass as bass
    import concourse.mybir as mybir
    import concourse.tile as tile
    from concourse import bacc
    from contextlib import ExitStack

    dt = mybir.dt
    Alu = mybir.AluOpType
    Act = mybir.ActivationFunctionType

    nc = bacc.Bacc("TRN2", target_bir_lowering=False, debug=False)

    # ---------------- DRAM I/O (host pre-laid in SBUF layout) ----------------
    def din(name, shape, dtype=dt.bfloat16):
        return nc.dram_tensor(name, list(shape), dtype, kind="ExternalInput").ap()

    x_own = din("x_own", [128, 6 * 1032])             # xT own chunk +-4 (padded)
    x_oth = din("x_oth", [128, 6 * 1024])             # xT other chunk
    wk = din("wk", [128, 6 * ALL_HEAD])
    wv = din("wv", [128, 6 * ALL_HEAD])
    wq = din("wq", [128, 6 * ALL_HEAD])
    wco = din("wco", [128, 6 * ALL_HEAD])
    wpw = din("wpw", [128, 6 * ALL_HEAD])
    wck = din("wck", [128, 3 * 54])
    sel = din("sel", [54, 6])                          # head-sum selector
    bvrow = din("bvrow", [1, ALL_HEAD])
    comask = din("comask", [1, 1032])
    bq = din("bq", [128, 3], dt.float32)
    bk = din("bk", [128, 3], dt.float32)
    convb = din("convb", [128, 3], dt.float32)
    bco = din("bco", [128, 3], dt.float32)
    bck = din("bck", [54, 1], dt.float32)
    dwsc = din("dwsc", [128, 6 * K], dt.float32)      # depthwise scalars
    dwd = din("dwd", [128, 3 * K * 128])              # diag dw mats (unused)

    attn = nc.dram_tensor("attn", [65, 6 * S], dt.bfloat16,
                          kind="ExternalOutput").ap()
    conv = nc.dram_tensor("conv", [128, 3 * CHUNK], dt.bfloat16,
                          kind="ExternalOutput").ap()
    pck_dram = nc.dram_tensor("pck_scratch", [54, CHUNK], dt.bfloat16).ap()
    den_dram = nc.dram_tensor("den_scratch", [6, CHUNK], dt.float32).ap()
    denb_dram = nc.dram_tensor("denb_scratch", [6, CHUNK], dt.bfloat16).ap()

    attn_r = attn.rearrange("p (h s) -> p h s", h=6)
    conv_r = conv.rearrange("p (a s) -> p a s", a=3)

    with tile.TileContext(nc) as tc, ExitStack() as ctx:
        singles = ctx.enter_context(tc.tile_pool(name="singles", bufs=1))
        persist = ctx.enter_context(tc.tile_pool(name="persist", bufs=1))
        work = ctx.enter_context(tc.tile_pool(name="work", bufs=3))

        def load(pool, src, shape, dtype=dt.bfloat16, name=None):
            t = pool.tile(shape, dtype, name=name)
            nc.sync.dma_start(out=t, in_=src)
            return t

        # ---------------- load inputs (issue order = priority) --------------
        xosb = singles.tile([128, 6, 1032], dt.bfloat16, name="xosb")
        xtsb = singles.tile([128, 6, 1024], dt.bfloat16, name="xtsb")
        wk_sb = singles.tile([128, 6, ALL_HEAD], dt.bfloat16, name="wk_sb")
        wv_sb = singles.tile([128, 6, ALL_HEAD], dt.bfloat16, name="wv_sb")
        wq_sb = singles.tile([128, 6, ALL_HEAD], dt.bfloat16, name="wq_sb")
        wco_sb = singles.tile([128, 6, ALL_HEAD], dt.bfloat16, name="wco_sb")
        wpw_sb = singles.tile([128, 6, ALL_HEAD], dt.bfloat16, name="wpw_sb")

        xo = x_own.rearrange("p (h s) -> p h s", h=6)
        xt = x_oth.rearrange("p (h s) -> p h s", h=6)
        wkr = wk.rearrange("p (h a) -> p h a", h=6)
        wvr = wv.rearrange("p (h a) -> p h a", h=6)
        wqr = wq.rearrange("p (h a) -> p h a", h=6)
        wcor = wco.rearrange("p (h a) -> p h a", h=6)
        wpwr = wpw.rearrange("p (h a) -> p h a", h=6)

        for dh in range(6):
            nc.sync.dma_start(out=wk_sb[:, dh], in_=wkr[:, dh])
            nc.sync.dma_start(out=xosb[:, dh], in_=xo[:, dh])
        bk_sb = load(singles, bk, [128, 3], dt.float32, name="bk_sb")
        for dh in range(6):
            nc.sync.dma_start(out=wv_sb[:, dh], in_=wvr[:, dh])
        bv_sb = load(singles, bvrow, [1, ALL_HEAD], name="bv_sb")
        for dh in range(6):
            nc.sync.dma_start(out=wq_sb[:, dh], in_=wqr[:, dh])
        bq_sb = load(singles, bq, [128, 3], dt.float32, name="bq_sb")
        dwsc_sb = load(singles, dwsc, [128, 6, K], dt.float32, name="dwsc_sb")
        for dh in range(6):
            nc.sync.dma_start(out=xtsb[:, dh], in_=xt[:, dh])
        for dh in range(6):
            nc.sync.dma_start(out=wco_sb[:, dh], in_=wcor[:, dh])
        bco_sb = load(singles, bco, [128, 3], dt.float32, name="bco_sb")
        mask_sb = singles.tile([128, 1032], dt.bfloat16, name="mask_sb")
        nc.sync.dma_start(out=mask_sb, in_=comask.to_broadcast([128, 1032]))
        for dh in range(6):
            nc.sync.dma_start(out=wpw_sb[:, dh], in_=wpwr[:, dh])
        convb_sb = load(singles, convb, [128, 3], dt.float32, name="convb_sb")
        wck_sb = load(singles, wck, [128, 3, 54], name="wck_sb")
        bck_sb = load(singles, bck, [54, 1], dt.float32, name="bck_sb")
        sel_sb = load(singles, sel, [54, 6], name="sel_sb")

        dwd_sb = singles.tile([128, 3, K, 128], dt.bfloat16, name="dwd_sb")
        nc.sync.dma_start(out=dwd_sb, in_=dwd)
        ones_sb = singles.tile([1, 128], dt.bfloat16, name="ones_sb")
        nc.gpsimd.memset(ones_sb, 1.0)

        # persistent intermediates
        qT = persist.tile([128, 3, S], dt.bfloat16, name="qT")
        kT = persist.tile([128, 3, CHUNK], dt.bfloat16, name="kT")
        vsb = persist.tile([128, 8, 6, 65], dt.bfloat16, name="vsb")
        dwT = persist.tile([128, 6, CHUNK], dt.bfloat16, name="dwT")
        caT = persist.tile([128, 3, CHUNK], dt.bfloat16, name="caT")
        coT = persist.tile([128, 3, 1032], dt.bfloat16, name="coT")
        accT = persist.tile([128, 3, CHUNK], dt.bfloat16, name="accT")
        pck = persist.tile([54, CHUNK], dt.bfloat16, name="pck")
        recipT = persist.tile([6, CHUNK], dt.float32, name="recipT")
        nc.gpsimd.memset(vsb[:, :, :, 64:65], 1.0)

        # PSUM pools: pj (projections) 2 banks, sc (scores) 4, cps (ctx) 2.
        pj = ctx.enter_context(tc.tile_pool(name="psum_pj", bufs=1,
                                            space="PSUM"))
        pa = ctx.enter_context(tc.tile_pool(name="psum_at", bufs=1,
                                            space="PSUM"))

        # HAM warm-up: ~6us of junk matmuls during the input DMA lead-in so
        # the PE clock gate is at 8/8 when the first real matmul issues.
        warm_ps = pj.tile([128, 32], dt.float32, tag="pj", bufs=2,
                          name="warm_ps")
        for _ in range(30):
            nc.tensor.matmul(warm_ps, ones_sb, ones_sb[:, 0:32],
                             start=True, stop=True)

        # ---------------- projection helpers (PE) ----------------------------
        def k_block(at):
            for sb in range(2):
                ps = pj.tile([128, 512], dt.float32, tag="pj", bufs=2,
                             name="pk")
                for dh in range(6):
                    nc.tensor.matmul(
                        ps, wk_sb[:, dh, at * 128:(at + 1) * 128],
                        xosb[:, dh, 4 + sb * 512: 4 + (sb + 1) * 512],
                        start=(dh == 0), stop=(dh == 5))
                nc.vector.tensor_scalar_add(
                    kT[:, at, sb * 512:(sb + 1) * 512], ps, bk_sb[:, at:at + 1])

        def v_block(st):
            pv = pj.tile([128, ALL_HEAD], dt.float32, tag="pj", bufs=2,
                         name="pv")
            for dh in range(6):
                nc.tensor.matmul(
                    pv, xosb[:, dh, 4 + st * 128: 4 + (st + 1) * 128],
                    wv_sb[:, dh, :], start=(dh == 0), stop=False)
            nc.tensor.matmul(pv, ones_sb, bv_sb, start=False, stop=True)
            nc.vector.tensor_copy(vsb[:, st, :, 0:64], pv.rearrange(
                "p (h d) -> p h d", h=6))

        def q_block(at, half, sb):
            # half 0: own chunk -> qT cols [0, 1024); half 1: other chunk.
            ps = pj.tile([128, 512], dt.float32, tag="pj", bufs=2, name="pq")
            for dh in range(6):
                if half == 0:
                    xs = xosb[:, dh, 4 + sb * 512: 4 + (sb + 1) * 512]
                else:
                    xs = xtsb[:, dh, sb * 512:(sb + 1) * 512]
                nc.tensor.matmul(
                    ps, wq_sb[:, dh, at * 128:(at + 1) * 128], xs,
                    start=(dh == 0), stop=(dh == 5))
            nc.vector.tensor_scalar_add(
                qT[:, at, half * 1024 + sb * 512: half * 1024 + (sb + 1) * 512],
                ps, bq_sb[:, at:at + 1])

        # co projection on chunk+-4 (1032 cols), bias + OOB mask on evac
        def co_block(at):
            for (o, w) in ((0, 512), (512, 512), (1024, 8)):
                pco = pj.tile([128, 512], dt.float32, tag="pj", bufs=2,
                              name="pco")
                for dh in range(6):
                    nc.tensor.matmul(
                        pco[:, :w], wco_sb[:, dh, at * 128:(at + 1) * 128],
                        xosb[:, dh, o:o + w],
                        start=(dh == 0), stop=(dh == 5))
                nc.vector.scalar_tensor_tensor(
                    out=coT[:, at, o:o + w], in0=pco[:, :w],
                    scalar=bco_sb[:, at:at + 1], in1=mask_sb[:, o:o + w],
                    op0=Alu.add, op1=Alu.mult)

        # pointwise conv fused with conv_attn: caT = (pw@dwT + convb) * qT
        def pw_block(at):
            for sb in range(2):
                ppw = pj.tile([128, 512], dt.float32, tag="pj", bufs=2,
                              name="ppw")
                for dh in range(6):
                    nc.tensor.matmul(
                        ppw, wpw_sb[:, dh, at * 128:(at + 1) * 128],
                        dwT[:, dh, sb * 512:(sb + 1) * 512],
                        start=(dh == 0), stop=(dh == 5))
                nc.vector.scalar_tensor_tensor(
                    out=caT[:, at, sb * 512:(sb + 1) * 512], in0=ppw,
                    scalar=convb_sb[:, at:at + 1],
                    in1=qT[:, at, sb * 512:(sb + 1) * 512],
                    op0=Alu.add, op1=Alu.mult)

        # depthwise conv: taps via tensor_scalar_mul on DVE (4x mode), add
        # tree on Pool (tensor_tensor, which Pool runs at DVE-2x speed).
        # scalar_tensor_tensor would be one pass per tap but runs at 1x.
        def dw_chain(ct):
            taps = []
            for k in range(K):
                tp = work.tile([128, CHUNK], dt.bfloat16, tag="dwtap", bufs=9,
                               name="dwtap")
                nc.vector.tensor_scalar_mul(
                    tp, xosb[:, ct, k:k + 1024], dwsc_sb[:, ct, k:k + 1])
                taps.append(tp)
            # add tree: 9 -> 5 -> 3 -> 2 -> 1 (last add lands in dwT)
            while len(taps) > 1:
                nxt = []
                for i in range(0, len(taps) - 1, 2):
                    dst = taps[i] if len(taps) > 2 else dwT[:, ct, :]
                    nc.vector.tensor_add(dst, taps[i], taps[i + 1])
                    nxt.append(dst)
                if len(taps) % 2:
                    nxt.append(taps[-1])
                taps = nxt

        # conv kernel layer -> exp -> DRAM (unnormalized; the softmax
        # denominator is divided out of accT at the end of the einsum, so
        # the span-weight broadcast does not wait on the recip roundtrip).
        rcbs = {}

        def ckl_all():
            for sb in range(2):
                pck_ps = pj.tile([54, 512], dt.float32, tag="pj", bufs=2,
                                 name="pck_ps")
                for at in range(3):
                    nc.tensor.matmul(
                        pck_ps, wck_sb[:, at, :],
                        caT[:, at, sb * 512:(sb + 1) * 512],
                        start=(at == 0), stop=(at == 2))
                nc.scalar.activation(pck[:, sb * 512:(sb + 1) * 512], pck_ps,
                                     Act.Exp, bias=bck_sb, scale=1.0)
            nc.sync.dma_start(out=pck_dram, in_=pck)
            for sb in range(2):
                pdt = pj.tile([6, 512], dt.float32, tag="pj", bufs=2,
                              name="pdt")
                nc.tensor.matmul(
                    pdt, sel_sb, pck[:, sb * 512:(sb + 1) * 512],
                    start=True, stop=True)
                nc.vector.reciprocal(recipT[:, sb * 512:(sb + 1) * 512], pdt)
            nc.sync.dma_start(out=den_dram, in_=recipT)
            recipb = persist.tile([6, CHUNK], dt.bfloat16, name="recipb")
            nc.vector.tensor_copy(recipb, recipT)
            nc.sync.dma_start(out=denb_dram, in_=recipb)
            for at in range(3):
                rcb = work.tile([128, CHUNK], dt.bfloat16, tag="rcb", bufs=3,
                                name="rcb")
                for hh in range(2):
                    srcap = bass.AP(
                        tensor=denb_dram.tensor,
                        offset=(2 * at + hh) * CHUNK,
                        ap=[[0, 64], [1, CHUNK]])
                    nc.sync.dma_start(out=rcb[hh * 64:(hh + 1) * 64],
                                      in_=srcap)
                rcbs[at] = rcb

        # span-weight broadcast tiles: 64-way partition broadcast from DRAM
        ckbs = {}

        def ckb_load(at, eng=None):
            eng = eng or nc.sync
            for k in range(K):
                ckb = work.tile([128, CHUNK], dt.bfloat16, tag="ckb", bufs=12,
                                name="ckb")
                for hh in range(2):
                    row = 18 * at + 9 * hh + k
                    srcap = bass.AP(
                        tensor=pck_dram.tensor,
                        offset=row * CHUNK,
                        ap=[[0, 64], [1, CHUNK]])
                    eng.dma_start(out=ckb[hh * 64:(hh + 1) * 64],
                                  in_=srcap)
                ckbs[(at, k)] = ckb

        # windowed einsum: accT[:,at,:] = sum_k ckb_k * coT[:,at,k:] as
        # tensor_tensor mul/add (2x on DVE; Pool takes tile 2).  Split into
        # halves so attention cps evacs interleave in the DVE FIFO.
        def einsum_nom(at, eng, half=None):
            tagsuf = "v" if eng is nc.vector else "g"
            ks = range(0, K) if half is None else (
                range(0, 5) if half == 0 else range(5, K))
            for k in ks:
                if k == 0:
                    eng.tensor_mul(accT[:, at, :], ckbs[(at, 0)],
                                   coT[:, at, 0:CHUNK])
                    continue
                tmp = work.tile([128, CHUNK], dt.bfloat16, tag="tmp" + tagsuf,
                                bufs=2, name="tmp")
                eng.tensor_mul(tmp, ckbs[(at, k)], coT[:, at, k:k + CHUNK])
                eng.tensor_add(accT[:, at, :], tmp, accT[:, at, :])

        def einsum_ship(at):
            nc.vector.tensor_mul(accT[:, at, :], accT[:, at, :], rcbs[at])
            for qr in range(2):
                nc.gpsimd.dma_start(
                    out=conv_r[:, at, qr * 512:(qr + 1) * 512],
                    in_=accT[:, at, qr * 512:(qr + 1) * 512])

        # ---------------- attention: one (head, qchunk) tile ------------------
        def attn_tile(h, qc, inject=None):
            at, lo = h // 2, (h % 2) * 64
            cps = pa.tile([65, 1024], dt.float32, tag="cps", bufs=1,
                          name="cps")
            for sk in range(8):
                if inject and sk in inject:
                    inject[sk]()
                sc = pa.tile([128, 1024], dt.float32, tag="sc", bufs=2,
                             name="sc")
                for qb in range(2):
                    nc.tensor.matmul(
                        sc[:, qb * 512:(qb + 1) * 512],
                        kT[lo:lo + 64, at, sk * 128:(sk + 1) * 128],
                        qT[lo:lo + 64, at,
                           qc * 1024 + qb * 512: qc * 1024 + (qb + 1) * 512],
                        start=True, stop=True)
                pt = work.tile([128, 1024], dt.bfloat16, tag="pt", bufs=4,
                               name="pt")
                nc.scalar.activation(pt, sc, Act.Exp, scale=0.125)
                for qb in range(2):
                    nc.tensor.matmul(
                        cps[:, qb * 512:(qb + 1) * 512], vsb[:, sk, h, :],
                        pt[:, qb * 512:(qb + 1) * 512],
                        start=(sk == 0), stop=(sk == 7))
            # evacuate numerator + denominator row (row 64) in bf16
            stg = work.tile([65, 1024], dt.bfloat16, tag="stg", bufs=6,
                            name="stg")
            nc.scalar.copy(stg, cps)
            for half in range(2):
                nc.sync.dma_start(
                    out=attn_r[:, h, qc * 1024 + half * 512:
                               qc * 1024 + (half + 1) * 512],
                    in_=stg[:, half * 512:(half + 1) * 512])

        # ---------------- emission schedule ----------------------------------
        # Conv branch work is spread between the early attention tiles (one
        # dw chain per tile keeps each tile's cps evac from queueing behind
        # bulk DVE work); the einsum drains mid-kernel on DVE+Pool so the
        # kernel tail is pure attention.
        k_block(0)
        for st in range(8):
            v_block(st)
        q_block(0, 0, 0)
        q_block(0, 0, 1)
        dw_chain(0)

        attn_tile(0, 0)
        k_block(1)
        q_block(1, 0, 0)
        q_block(1, 0, 1)
        dw_chain(1)
        attn_tile(1, 0)
        k_block(2)
        q_block(2, 0, 0)
        q_block(2, 0, 1)
        dw_chain(2)
        attn_tile(2, 0)
        q_block(0, 1, 0)
        q_block(0, 1, 1)
        dw_chain(3)
        attn_tile(3, 0)
        q_block(1, 1, 0)
        q_block(1, 1, 1)
        dw_chain(4)
        co_block(0)
        attn_tile(4, 0)
        q_block(2, 1, 0)
        q_block(2, 1, 1)
        dw_chain(5)
        co_block(1)
        attn_tile(5, 0)
        co_block(2)
        pw_block(0)
        pw_block(1)
        pw_block(2)
        ckl_all()
        ckb_load(2)
        einsum_nom(2, nc.gpsimd)
        einsum_ship(2)
        attn_tile(0, 1)
        ckb_load(0)
        attn_tile(1, 1)
        einsum_nom(0, nc.vector)
        einsum_ship(0)
        ckb_load(1)
        attn_tile(2, 1)
        einsum_nom(1, nc.vector)
        einsum_ship(1)
        attn_tile(3, 1)
        attn_tile(4, 1)
        attn_tile(5, 1)

    nc.compile()
    return nc


def _prep_in_maps(inputs):
    x = np.asarray(inputs["x"], np.float32)
    dw = np.asarray(inputs["dw"], np.float32).reshape(HIDDEN, K)

    def sb_layout(wT, ntile):  # [ntile*128, F] -> [128, ntile*F]
        f = wT.shape[1]
        return np.ascontiguousarray(
            wT.reshape(ntile, 128, f).transpose(1, 0, 2).reshape(128, ntile * f))

    def wprep(w):  # [A, HIDDEN] -> bf16 [128, 6*A]
        return sb_layout(np.ascontiguousarray(w.T).astype(BF16), 6)

    # depthwise scalars: dwsc[c', ct*9+k] = dw[ct*128+c', k]
    dwsc = np.ascontiguousarray(
        dw.reshape(6, 128, K).transpose(1, 0, 2).reshape(128, 6 * K),
        np.float32)

    dwdm = np.zeros((128, 3, K, 128), BF16)
    ii = np.arange(128)
    for ct in (3, 4, 5):
        for k in range(K):
            dwdm[ii, ct - 3, k, ii] = dw[ct * 128 + ii, k].astype(BF16)

    com = {
        "wq": wprep(inputs["Wq"]), "wk": wprep(inputs["Wk"]),
        "wv": wprep(inputs["Wv"]), "wco": wprep(inputs["Wco"]),
        "wpw": wprep(inputs["pw"]),
        "wck": sb_layout(np.ascontiguousarray(inputs["Wck"].T).astype(BF16), 3),
        "sel": np.kron(np.eye(N_HEADS), np.ones((K, 1))).astype(BF16),
        "bvrow": inputs["bv"].reshape(1, ALL_HEAD).astype(BF16),
        "bq": np.ascontiguousarray(inputs["bq"].reshape(3, 128).T, np.float32),
        "bk": np.ascontiguousarray(inputs["bk"].reshape(3, 128).T, np.float32),
        "convb": np.ascontiguousarray(
            inputs["conv_bias"].reshape(3, 128).T, np.float32),
        "bco": np.ascontiguousarray(inputs["bco"].reshape(3, 128).T, np.float32),
        "bck": inputs["bck"].reshape(54, 1).astype(np.float32),
        "dwsc": dwsc,
        "dwd": dwdm.reshape(128, 3 * K * 128),
    }

    in_maps = []
    for b in range(B):
        xb = x[b]                                   # [S, HIDDEN]
        xTb = np.ascontiguousarray(xb.T).astype(BF16)   # [768, S]
        xT_pad = np.zeros((HIDDEN, S + 8), BF16)
        xT_pad[:, 4:4 + S] = xTb
        for j in range(2):
            own = np.ascontiguousarray(xT_pad[:, j * CHUNK: j * CHUNK + 1032])
            oth = np.ascontiguousarray(
                xTb[:, (1 - j) * CHUNK: (2 - j) * CHUNK])
            g0 = j * CHUNK - 4
            mrows = np.arange(g0, g0 + 1032)
            comask = ((mrows >= 0) & (mrows < S)).astype(BF16).reshape(1, 1032)
            m = dict(com)
            m["x_own"] = sb_layout(own, 6)
            m["x_oth"] = sb_layout(oth, 6)
            m["comask"] = comask
            in_maps.append(m)
    return in_maps


def _gather(results):
    out = np.empty((B, S, 2 * ALL_HEAD), np.float32)
    for b in range(B):
        r0, r1 = results[2 * b], results[2 * b + 1]
        # attn partials: [65, 6, 2048] own-chunk-first; row 64 = denominator
        a0 = np.asarray(r0["attn"]).astype(np.float32).reshape(65, 6, S)
        a1 = np.asarray(r1["attn"]).astype(np.float32).reshape(65, 6, S)
        n0, d0 = a0[:64], a0[64]
        n1, d1 = a1[:64], a1[64]
        # core 1's columns are [own=chunk1 | other=chunk0]: swap to global
        n1 = np.concatenate([n1[:, :, CHUNK:], n1[:, :, :CHUNK]], axis=2)
        d1 = np.concatenate([d1[:, CHUNK:], d1[:, :CHUNK]], axis=1)
        ctx = (n0 + n1) / (d0 + d1)[None]            # [64, 6, 2048]
        out[b, :, :ALL_HEAD] = ctx.transpose(2, 1, 0).reshape(S, ALL_HEAD)
        for j in range(2):
            cv = np.asarray(results[2 * b + j]["conv"]).astype(np.float32)
            cv = cv.reshape(128, 3, CHUNK).transpose(2, 1, 0).reshape(
                CHUNK, ALL_HEAD)
            out[b, j * CHUNK:(j + 1) * CHUNK, ALL_HEAD:] = cv
    return out


def kernel(**inputs):
    from concourse.bass_utils import run_bass_kernel_spmd

    key = "prog"
    if key not in _COMPILED:
        _COMPILED[key] = _build_program()
    nc = _COMPILED[key]
    in_maps = _prep_in_maps(inputs)
    res = run_bass_kernel_spmd(nc, in_maps, list(range(N_CORES)))
    return _gather(res.results)


if __name__ == "__main__":
    import reference
    inp = {k: np.asarray(v) for k, v in reference.setup_inputs().items()}
    got = kernel(**inp)
    want = np.asarray(reference.reference(**inp))
    err = np.linalg.norm(got - want) / np.linalg.norm(want)
    print("rel err:", err)
